# revision 32
# baseline (speedup 1.0000x reference)
"""Trainium2 Bass kernel for fused sparse attention (policy-masked softmax).

Computation (per batch b):
    qkv  = x @ qkv_w.T + qkv_b                  -> q, k, v   [H heads, hd=64]
    S    = (q @ k.T) * hd**-0.5                 [H, N, N]
    P    = eps-softmax(S) with key-policy mask and eye-blend
    out  = (P @ v) @ proj_w.T + proj_b

Strategy: pure data-parallel over batch across 8 NeuronCores (4 batches
per core), fully fused on-chip per batch.  Host pre-transposes x and the
weights so the device kernel needs no transposes:
  - x^T [C, N] tiles are the shared lhsT/rhs for the QKV projections
  - q^T/k^T land as [64, N] head slices (contraction dim on partitions)
  - softmax runs in the S^T [key, query] orientation: the policy mask is a
    per-partition scalar, the attn row-sum rides along the attn@v matmul
    via a per-head all-ones lhsT column in v_ext, and 1/sum is applied via
    reciprocal_approx_fast + gpsimd partition-broadcast.
  - attn output accumulates directly in proj-ready [C, N] layout.

Scheduling: per-head software pipeline.  PSUM banks are partitioned into
three dedicated pools (S: 4, attn@v: 2, qkv/proj chains: 2) so projection
"filler" matmuls never wait on softmax evictions.  Each head iteration
emits: exp+mask (ACT / DVE+GpSimd), next head's S matmuls, ~2 filler
chains (qkv projections of batch b+1 and output projection of batch b-1),
then the attn@v matmuls.  This keeps the PE fed during the softmax
element-wise chain and removes the batch-boundary stalls.
Matmul operands are fp16 (1 cycle/row, fp32 PSUM accumulation).
"""

import sys

if "/opt/trn_rl_repo" not in sys.path:
    sys.path.insert(0, "/opt/trn_rl_repo")

import numpy as np

B, N, C, H = 32, 384, 768, 12
HD = C // H  # 64
NCORES = 8
BL = B // NCORES  # batches per core
EPS = 1e-6
SCALE = HD ** -0.5
P = 128
KT = C // P   # 6 contraction tiles over C
NT = N // P   # 3 tiles over sequence
VS = 128      # per-head v stride in v_ext: [ones | 63 junk | v(64)]
VOFF = 64     # v offset within a head's block (psum reads need 64-aligned base)
JQK = 2 * C // P  # 12 q/k output tiles

_CACHE = {}


def _build_nc():
    import concourse.tile as tile
    from concourse import bacc, mybir

    F32 = mybir.dt.float32
    F16 = mybir.dt.float16
    EXP = mybir.ActivationFunctionType.Exp
    IDENT = mybir.ActivationFunctionType.Identity
    MULT = mybir.AluOpType.mult
    ADD = mybir.AluOpType.add
    NE = mybir.AluOpType.not_equal

    nc = bacc.Bacc(None, target_bir_lowering=False)

    xT_d = nc.declare_dram_parameter("xT", [BL, P, KT, N], F16, isOutput=False)
    pol_d = nc.declare_dram_parameter("pol", [BL, P, NT], F32, isOutput=False)
    # weights packed host-side as 8 kt-major sections of [P, KT, 384] so a
    # section loads with one DMA of ~4.6KB contiguous runs per partition:
    # 0: wq[0:384]  1: wk[0:384]  2: wq[384:]  3: wk[384:]
    # 4: wv[0:384]  5: wv[384:]   6: wp[0:384] 7: wp[384:]
    wall_d = nc.declare_dram_parameter("wall", [P, 8, KT, 384], F16,
                                       isOutput=False)
    # bqk | bv | bp broadcast rows packed into one f32 tensor
    meta_d = nc.declare_dram_parameter("meta", [P, JQK + 2 * C], F32,
                                       isOutput=False)
    out_d = nc.declare_dram_parameter("out", [BL, N, C], F32, isOutput=True)


    with tile.TileContext(nc) as tc:
        with (
            tc.tile_pool(name="singles", bufs=1) as singles,
            tc.tile_pool(name="xin", bufs=BL) as xin,
            tc.tile_pool(name="mid", bufs=3) as mid,
            tc.tile_pool(name="eact", bufs=3) as eact,
            tc.tile_pool(name="ehatp", bufs=3) as ehatp,
            tc.tile_pool(name="small", bufs=6) as small,
            tc.tile_pool(name="outp", bufs=2) as outp,
            tc.tile_pool(name="psS", bufs=3, space="PSUM") as psS,
            tc.tile_pool(name="psQ", bufs=3, space="PSUM") as psQ,
            tc.tile_pool(name="psAV", bufs=2, space="PSUM") as psAV,
        ):
            # ---- tiny tensors + batch 0 inputs + wq first so qk(b=0) can
            # start ASAP.  DMA issue is spread over several engine queues:
            # each dma_start costs the issuing sequencer ~0.6us.
            meta_sb = singles.tile([P, JQK + 2 * C], F32)
            bqk_sb = meta_sb[:, 0:JQK]
            bv_sb = meta_sb[:, JQK : JQK + C]
            bp_sb = meta_sb[:, JQK + C : JQK + 2 * C]
            pol_sbs = [xin.tile([P, NT], F32, tag="pol", name=f"pol{b}")
                       for b in range(BL)]
            xT_sbs = [xin.tile([P, KT, N], F16, tag="xT", name=f"xT{b}")
                      for b in range(BL)]
            nc.gpsimd.dma_start(out=xT_sbs[0], in_=xT_d[0])
            nc.gpsimd.dma_start(
                out=meta_sb[:, 0:JQK], in_=meta_d[:, 0:JQK]
            )

            def declare_w(name):
                return [singles.tile([P, KT, 384], F16, tag=f"{name}{i}",
                                     name=f"{name}{i}") for i in range(2)]

            def dma_w(eng, tiles, half, secs):
                eng.dma_start(out=tiles[half], in_=wall_d[:, secs[half]])

            wq_t = declare_w("wq")
            wk_t = declare_w("wk")
            wv_t = declare_w("wv")
            wp_t = declare_w("wp")
            dma_w(nc.sync, wq_t, 0, (0, 2))
            dma_w(nc.scalar, wk_t, 0, (1, 3))
            dma_w(nc.sync, wq_t, 1, (0, 2))
            dma_w(nc.gpsimd, wk_t, 1, (1, 3))
            nc.sync.dma_start(out=pol_sbs[0], in_=pol_d[0])
            # dummy exp pulls the one-time ACT table load off the critical path
            warm = singles.tile([1, 1], F32)
            nc.vector.memset(warm, 0.0)
            nc.scalar.activation(out=warm, in_=warm, func=EXP, scale=1.0)

            # PE pre-warm: matmuls on uninitialized SBUF garbage (psum never
            # read) keep the HAM clock-gate at 8/8 while the prologue DMAs
            # land, so the first real chains stream at 2.4GHz
            dummy = singles.tile([P, 512], F16, tag="dummy")
            nc.vector.memset(dummy, 0.0)
            for i in range(30):
                ps_w = psQ.tile([P, 512], F32, tag="q", name=f"warm{i}")
                nc.tensor.matmul(
                    ps_w, dummy[:, 0:P], dummy, start=True, stop=True,
                )

            for b in range(1, BL):
                nc.gpsimd.dma_start(out=pol_sbs[b], in_=pol_d[b])

            # ---- persistent v_ext buffers: [ones | v(64)] per head;
            # ones written once, v(64) regions overwritten per batch
            v_exts = []
            for i in range(2):
                ve = singles.tile([P, NT, H * VS], F16, tag=f"ve{i}")
                vv = ve.rearrange("p t (h s) -> p t h s", s=VS)
                # cols [1, VOFF) are never read back from psum -> no zeroing
                nc.vector.memset(vv[:, :, :, 0:1], 1.0)
                v_exts.append(ve)

            # ---- blend tiles: blend[p, t, m] = 1 if m == t*128+p else pol[p]
            # Built lazily (one tile at a time, interleaved into the prior
            # batch's iterations) so the scalar FIFO stays clear early on.
            blends = {}

            def build_blend(b, t):
                if b not in blends:
                    blends[b] = xin.tile([P, NT, N], F16, tag="blend",
                                         name=f"bl{b}")
                blend = blends[b]
                # input is only a shape-provider (scale=0): use xT_sbs[0]
                # which is always loaded first
                nc.scalar.activation(
                    out=blend[:, t, :], in_=xT_sbs[0][:, 0, :],
                    func=IDENT, bias=pol_sbs[b][:, t : t + 1], scale=0.0,
                )
                nc.gpsimd.affine_select(
                    out=blend[:, t, :], in_=blend[:, t, :],
                    compare_op=NE, fill=1.0, base=t * P,
                    pattern=[[-1, N]], channel_multiplier=1,
                )

            # ================= emitters =================
            qkTs = {}   # (b, jt) -> tile

            def qk_chain(b, jt):
                t = mid.tile([P, N], F16, tag=f"qkT{jt}", name=f"qk{b}_{jt}")
                qkTs[(b, jt)] = t
                ps = psQ.tile([P, 512], F32, tag="q")
                half = wq_t if jt < JQK // 2 else wk_t
                joff = (jt % (JQK // 2)) * P
                wtile = half[joff // 384]
                for kt in range(KT):
                    nc.tensor.matmul(
                        ps[:, :N],
                        wtile[:, kt, joff % 384 : joff % 384 + P],
                        xT_sbs[b][:, kt, :],
                        start=(kt == 0), stop=(kt == KT - 1),
                    )
                # bias add + fp16 round (psum -> sbuf), alternating engines
                if jt % 2 == 0:
                    nc.scalar.activation(
                        out=t, in_=ps[:, :N],
                        func=IDENT, bias=bqk_sb[:, jt : jt + 1], scale=1.0,
                    )
                else:
                    nc.vector.tensor_scalar(
                        out=t, in0=ps[:, :N],
                        scalar1=bqk_sb[:, jt : jt + 1], scalar2=None,
                        op0=ADD,
                    )

            def v_chain(b, i):
                nt, c0 = i // 2, (i % 2) * 384
                cw = 384
                v_ext = v_exts[b % 2]
                v_dst = v_ext.rearrange("p t (h s) -> p t h s", s=VS)
                ps = psQ.tile([P, 512], F32, tag="q")
                for kt in range(KT):
                    nc.tensor.matmul(
                        ps[:, :cw],
                        xT_sbs[b][:, kt, nt * P : (nt + 1) * P],
                        wv_t[c0 // 384][:, kt, :],
                        start=(kt == 0), stop=(kt == KT - 1),
                    )
                h0, hn = c0 // HD, cw // HD
                nc.vector.tensor_tensor(
                    out=v_dst[:, nt, h0 : h0 + hn, VOFF : VOFF + HD],
                    in0=ps[:, :cw].rearrange("p (h d) -> p h d", d=HD),
                    in1=bv_sb[:, c0 : c0 + cw].rearrange(
                        "p (h d) -> p h d", d=HD
                    ),
                    op=ADD,
                )

            def s_head(b, h):
                # S^T matmuls for head h -> 3 psum tiles [128 keys, 384 q]
                jq, jk = h // 2, JQK // 2 + h // 2
                base = (h % 2) * HD
                qh = qkTs[(b, jq)][base : base + HD, :]
                kh = qkTs[(b, jk)][base : base + HD, :]
                out = []
                for mt in range(NT):
                    ps_s = psS.tile([P, 512], F32, tag="s")
                    nc.tensor.matmul(
                        ps_s[:, :N],
                        kh[:, mt * P : (mt + 1) * P],
                        qh,
                        start=True, stop=True,
                    )
                    out.append(ps_s)
                return out

            def exp_blend(b, h, ps_ss):
                # ea = exp(S*scale) into one fused [P, NT, N] tile, then a
                # single wide DVE multiply applies the blend mask
                ea = eact.tile([P, NT, N], F16, tag="ea", name=f"ea{h}")
                for mt in range(NT):
                    nc.scalar.activation(
                        out=ea[:, mt, :], in_=ps_ss[mt][:, :N],
                        func=EXP, scale=SCALE,
                    )
                eh = ehatp.tile([P, NT, N], F16, tag="eh", name=f"eh{h}")
                nc.vector.tensor_tensor(
                    out=eh, in0=ea, in1=blends[b], op=MULT,
                )
                return eh

            def av_head(b, h, ehat):
                v_ext = v_exts[b % 2]
                ps_av = psAV.tile([P, 512], F32, tag="av", name=f"av{h}")
                for mt in range(NT):
                    nc.tensor.matmul(
                        ps_av[: VOFF + HD, :N],
                        v_ext[:, mt, h * VS : (h + 1) * VS],
                        ehat[:, mt, :],
                        start=(mt == 0), stop=(mt == NT - 1),
                    )
                return ps_av

            def norm_head(h, ps_av, oT):
                # row-sum sits in psum partition 0 via the ones column;
                # 1/sum read directly from psum (saves an ACT copy)
                base = (h % 2) * HD
                jq = h // 2
                r_sb = small.tile([1, N], F32, tag="r", name=f"r{h}")
                nc.vector.reciprocal_approx_fast(out=r_sb, in_=ps_av[0:1, :N])
                rb_sb = small.tile([HD, N], F32, tag="rb", name=f"rb{h}")
                nc.gpsimd.partition_broadcast(rb_sb, r_sb)
                nc.vector.tensor_tensor(
                    out=oT[jq][base : base + HD, :],
                    in0=ps_av[VOFF : VOFF + HD, :N], in1=rb_sb, op=MULT,
                )

            def proj_chain(b, i, oT):
                nt, c0 = i // 2, (i % 2) * 384
                cw = 384
                out_sb = proj_sbs[b % 2]
                out_v = out_d[b].rearrange("(t p) o -> p t o", p=P)
                ps = psQ.tile([P, 512], F32, tag="q")
                for kt in range(KT):
                    nc.tensor.matmul(
                        ps[:, :cw],
                        oT[kt][:, nt * P : (nt + 1) * P],
                        wp_t[c0 // 384][:, kt, :],
                        start=(kt == 0), stop=(kt == KT - 1),
                    )
                nc.vector.tensor_add(
                    out=out_sb[:, nt, c0 : c0 + cw],
                    in0=ps[:, :cw],
                    in1=bp_sb[:, c0 : c0 + cw],
                )
                eng = (nc.sync, nc.scalar, nc.gpsimd)[nt]
                eng.dma_start(
                    out=out_v[:, nt : nt + 1, c0 : c0 + cw],
                    in_=out_sb[:, nt : nt + 1, c0 : c0 + cw],
                )

            proj_sbs = [outp.tile([P, NT, C], F32, tag="out", name=f"os{i}")
                        for i in range(2)]

            # ================= schedule =================
            # prologue: batch 0's qk and v chains, with the bulk input
            # transfers issued once the critical loads are in flight
            nc.gpsimd.dma_start(out=xT_sbs[1], in_=xT_d[1])
            for jt in (0, 6, 1, 7, 2, 8, 3, 9):
                qk_chain(0, jt)
                if jt == 6:
                    dma_w(nc.scalar, wv_t, 0, (4, 5))
                    dma_w(nc.scalar, wv_t, 1, (4, 5))
            for t in range(NT):
                build_blend(0, t)
            nc.gpsimd.dma_start(
                out=meta_sb[:, JQK:], in_=meta_d[:, JQK:]
            )
            for i in (0, 2, 4):
                v_chain(0, i)

            oTs = {}

            def make_oT(b):
                oTs[b] = [
                    mid.tile([P, N], F16, tag=f"oT{kt}", name=f"oT{b}_{kt}")
                    for kt in range(KT)
                ]

            make_oT(0)
            pending = s_head(0, 0)
            late = None

            for b in range(BL):
                oT = oTs[b]
                # filler chains, placed by deadline: this batch's deferred
                # tail chains first, then b-1's output projection, then the
                # head chains of b+1.  This keeps the last batch supplied
                # with PE work instead of front-loading everything.
                per_iter = [[] for _ in range(H)]
                per_iter[0] = [("qk", b, 4), ("qk", b, 10)]
                per_iter[1] = [("qk", b, 5), ("qk", b, 11)]
                per_iter[2] = [("v", b, 1)]
                per_iter[3] = [("v", b, 3)]
                per_iter[4] = [("v", b, 5)]
                if b > 0:
                    for i, it in enumerate((2, 3, 4, 5, 6, 7)):
                        per_iter[it].append(("p", b - 1, i))
                if b + 1 < BL:
                    per_iter[5].append(("qk", b + 1, 0))
                    per_iter[6].append(("qk", b + 1, 6))
                    per_iter[7].append(("qk", b + 1, 1))
                    per_iter[8] += [("qk", b + 1, 7), ("qk", b + 1, 2)]
                    per_iter[9] += [("qk", b + 1, 8), ("qk", b + 1, 3)]
                    per_iter[10] += [("qk", b + 1, 9), ("v", b + 1, 0)]
                    per_iter[11] += [("v", b + 1, 2), ("v", b + 1, 4)]

                def emit_filler(f):
                    kind, fb, fi = f
                    if kind == "qk":
                        qk_chain(fb, fi)
                    elif kind == "v":
                        v_chain(fb, fi)
                    else:
                        proj_chain(fb, fi, oTs[fb])

                for h in range(H):
                    # stagger the remaining bulk loads through batch 0
                    if b == 0:
                        if h == 1:
                            nc.sync.dma_start(out=xT_sbs[2], in_=xT_d[2])
                        elif h == 3:
                            dma_w(nc.sync, wp_t, 0, (6, 7))
                            dma_w(nc.sync, wp_t, 1, (6, 7))
                        elif h == 5:
                            nc.sync.dma_start(out=xT_sbs[3], in_=xT_d[3])
                    # build next batch's blend tiles mid-batch
                    if b + 1 < BL and h in (3, 5, 7):
                        build_blend(b + 1, (h - 3) // 2)
                    # last batch is latency-paced: put the previous head's
                    # norm at the DVE FIFO head (ahead of the blend, which
                    # waits ~1.7us of exps) so the AV psum recycles early
                    if b == BL - 1 and late:
                        norm_head(*late)
                        late = None
                    # 1) softmax element-wise ops for head h (ACT + one DVE)
                    ehat = exp_blend(b, h, pending)
                    # 2) next head's S matmuls (PE; psS pool)
                    if h + 1 < H:
                        pending = s_head(b, h + 1)
                    elif b + 1 < BL:
                        make_oT(b + 1)
                        pending = s_head(b + 1, 0)
                    # 3) filler chains (PE; psQ pool) hide the ehat latency
                    fl = per_iter[h]
                    if fl:
                        emit_filler(fl[0])
                    # 4) attn@v matmuls for head h (PE; psAV pool)
                    ps_av = av_head(b, h, ehat)
                    # 5) late softmax stages of the PREVIOUS head (1-head
                    #    skew keeps recip/rnorm waits off the FIFO heads);
                    #    no skew at the very end to shorten the tail
                    if late:
                        norm_head(*late)
                    if b == BL - 1 and h == H - 1:
                        norm_head(h, ps_av, oT)
                        late = None
                    else:
                        late = (h, ps_av, oT)
                    for f in fl[1:]:
                        emit_filler(f)

            if late:
                norm_head(*late)
            # final batch's output projection
            for i in range(6):
                proj_chain(BL - 1, i, oTs[BL - 1])

    nc.compile()
    return nc


def _get_nc():
    if "nc" not in _CACHE:
        _CACHE["nc"] = _build_nc()
    return _CACHE["nc"]


def kernel(x, policy, qkv_w, qkv_b, proj_w, proj_b):
    from concourse.bass_utils import run_bass_kernel_spmd

    nc = _get_nc()

    x = np.asarray(x, dtype=np.float32)
    policy = np.asarray(policy, dtype=np.float32)
    qkv_w = np.asarray(qkv_w, dtype=np.float32)
    qkv_b = np.asarray(qkv_b, dtype=np.float32)
    proj_w = np.asarray(proj_w, dtype=np.float32)
    proj_b = np.asarray(proj_b, dtype=np.float32)

    xT = np.ascontiguousarray(
        x.transpose(0, 2, 1).reshape(B, KT, P, N).transpose(0, 2, 1, 3)
    ).astype(np.float16)  # [B, P, KT, N]
    pol = np.ascontiguousarray(
        policy.reshape(B, N).reshape(B, NT, P).transpose(0, 2, 1)
    )  # [B, P, NT]

    def to_sbuf_layout(w):  # [C, J] -> [P, KT, J]
        return np.ascontiguousarray(w.reshape(KT, P, -1).transpose(1, 0, 2))

    wqkT = to_sbuf_layout(qkv_w[: 2 * C].T.astype(np.float16))
    wpT = to_sbuf_layout(proj_w.T.astype(np.float16))
    bqk = np.ascontiguousarray(qkv_b[: 2 * C].reshape(JQK, P).T)  # [P, 12]
    wvT = to_sbuf_layout(qkv_w[2 * C :].T.astype(np.float16))
    # 8 kt-major sections [P, KT, 384]: wq0 wk0 wq1 wk1 wv0 wv1 wp0 wp1
    wall = np.ascontiguousarray(np.stack([
        wqkT[:, :, 0:384], wqkT[:, :, 768:1152],
        wqkT[:, :, 384:768], wqkT[:, :, 1152:1536],
        wvT[:, :, 0:384], wvT[:, :, 384:768],
        wpT[:, :, 0:384], wpT[:, :, 384:768],
    ], axis=1))  # [P, 8, KT, 384]
    # bqk | bv | bp broadcast rows in one f32 tensor [P, 12 + 2C]
    meta = np.ascontiguousarray(np.concatenate([
        bqk,
        np.broadcast_to(qkv_b[2 * C :], (P, C)),
        np.broadcast_to(proj_b, (P, C)),
    ], axis=1).astype(np.float32))

    in_maps = []
    for c in range(NCORES):
        s = slice(c * BL, (c + 1) * BL)
        in_maps.append({
            "xT": xT[s], "pol": pol[s],
            "wall": wall, "meta": meta,
        })

    res = run_bass_kernel_spmd(nc, in_maps, core_ids=list(range(NCORES)))
    _CACHE["last_results"] = res
    out = np.concatenate(
        [res.results[c]["out"] for c in range(NCORES)], axis=0
    ).astype(np.float32)
    return out


# revision 33
# speedup vs baseline: 1.0129x; 1.0129x over previous
"""Trainium2 Bass kernel for fused sparse attention (policy-masked softmax).

Computation (per batch b):
    qkv  = x @ qkv_w.T + qkv_b                  -> q, k, v   [H heads, hd=64]
    S    = (q @ k.T) * hd**-0.5                 [H, N, N]
    P    = eps-softmax(S) with key-policy mask and eye-blend
    out  = (P @ v) @ proj_w.T + proj_b

Strategy: pure data-parallel over batch across 8 NeuronCores (4 batches
per core), fully fused on-chip per batch.  Host pre-transposes x and the
weights so the device kernel needs no transposes:
  - x^T [C, N] tiles are the shared lhsT/rhs for the QKV projections
  - q^T/k^T land as [64, N] head slices (contraction dim on partitions)
  - softmax runs in the S^T [key, query] orientation: the policy mask is a
    per-partition scalar, the attn row-sum rides along the attn@v matmul
    via a per-head all-ones lhsT column in v_ext, and 1/sum is applied via
    reciprocal_approx_fast + gpsimd partition-broadcast.
  - attn output accumulates directly in proj-ready [C, N] layout.

Scheduling: per-head software pipeline.  PSUM banks are partitioned into
three dedicated pools (S: 4, attn@v: 2, qkv/proj chains: 2) so projection
"filler" matmuls never wait on softmax evictions.  Each head iteration
emits: exp+mask (ACT / DVE+GpSimd), next head's S matmuls, ~2 filler
chains (qkv projections of batch b+1 and output projection of batch b-1),
then the attn@v matmuls.  This keeps the PE fed during the softmax
element-wise chain and removes the batch-boundary stalls.
Matmul operands are fp16 (1 cycle/row, fp32 PSUM accumulation).
"""

import sys

if "/opt/trn_rl_repo" not in sys.path:
    sys.path.insert(0, "/opt/trn_rl_repo")

import numpy as np

B, N, C, H = 32, 384, 768, 12
HD = C // H  # 64
NCORES = 8
BL = B // NCORES  # batches per core
EPS = 1e-6
SCALE = HD ** -0.5
P = 128
KT = C // P   # 6 contraction tiles over C
NT = N // P   # 3 tiles over sequence
VS = 128      # per-head v stride in v_ext: [ones | 63 junk | v(64)]
VOFF = 64     # v offset within a head's block (psum reads need 64-aligned base)
JQK = 2 * C // P  # 12 q/k output tiles

_CACHE = {}


def _build_nc():
    import concourse.tile as tile
    from concourse import bacc, mybir

    F32 = mybir.dt.float32
    F16 = mybir.dt.float16
    EXP = mybir.ActivationFunctionType.Exp
    IDENT = mybir.ActivationFunctionType.Identity
    MULT = mybir.AluOpType.mult
    ADD = mybir.AluOpType.add
    NE = mybir.AluOpType.not_equal

    nc = bacc.Bacc(None, target_bir_lowering=False)

    xT_d = nc.declare_dram_parameter("xT", [BL, P, KT, N], F16, isOutput=False)
    pol_d = nc.declare_dram_parameter("pol", [BL, P, NT], F32, isOutput=False)
    # weights packed host-side as 8 kt-major sections of [P, KT, 384] so a
    # section loads with one DMA of ~4.6KB contiguous runs per partition:
    # 0: wq[0:384]  1: wk[0:384]  2: wq[384:]  3: wk[384:]
    # 4: wv[0:384]  5: wv[384:]   6: wp[0:384] 7: wp[384:]
    wall_d = nc.declare_dram_parameter("wall", [P, 8, KT, 384], F16,
                                       isOutput=False)
    # bqk | bv | bp broadcast rows packed into one f32 tensor
    meta_d = nc.declare_dram_parameter("meta", [P, JQK + 2 * C], F32,
                                       isOutput=False)
    out_d = nc.declare_dram_parameter("out", [BL, N, C], F32, isOutput=True)


    with tile.TileContext(nc) as tc:
        with (
            tc.tile_pool(name="singles", bufs=1) as singles,
            tc.tile_pool(name="xin", bufs=BL) as xin,
            tc.tile_pool(name="mid", bufs=3) as mid,
            tc.tile_pool(name="eact", bufs=3) as eact,
            tc.tile_pool(name="ehatp", bufs=3) as ehatp,
            tc.tile_pool(name="small", bufs=6) as small,
            tc.tile_pool(name="outp", bufs=2) as outp,
            tc.tile_pool(name="psS", bufs=3, space="PSUM") as psS,
            tc.tile_pool(name="psQ", bufs=3, space="PSUM") as psQ,
            tc.tile_pool(name="psAV", bufs=2, space="PSUM") as psAV,
        ):
            # ---- tiny tensors + batch 0 inputs + wq first so qk(b=0) can
            # start ASAP.  DMA issue is spread over several engine queues:
            # each dma_start costs the issuing sequencer ~0.6us.
            meta_sb = singles.tile([P, JQK + 2 * C], F32)
            bqk_sb = meta_sb[:, 0:JQK]
            bv_sb = meta_sb[:, JQK : JQK + C]
            bp_sb = meta_sb[:, JQK + C : JQK + 2 * C]
            pol_sbs = [xin.tile([P, NT], F32, tag="pol", name=f"pol{b}")
                       for b in range(BL)]
            xT_sbs = [xin.tile([P, KT, N], F16, tag="xT", name=f"xT{b}")
                      for b in range(BL)]
            nc.gpsimd.dma_start(out=xT_sbs[0], in_=xT_d[0])
            nc.gpsimd.dma_start(
                out=meta_sb[:, 0:JQK], in_=meta_d[:, 0:JQK]
            )

            def declare_w(name):
                return [singles.tile([P, KT, 384], F16, tag=f"{name}{i}",
                                     name=f"{name}{i}") for i in range(2)]

            def dma_w(eng, tiles, half, secs):
                eng.dma_start(out=tiles[half], in_=wall_d[:, secs[half]])

            wq_t = declare_w("wq")
            wk_t = declare_w("wk")
            wv_t = declare_w("wv")
            wp_t = declare_w("wp")
            dma_w(nc.sync, wq_t, 0, (0, 2))
            dma_w(nc.scalar, wk_t, 0, (1, 3))
            dma_w(nc.sync, wq_t, 1, (0, 2))
            dma_w(nc.scalar, wk_t, 1, (1, 3))
            nc.sync.dma_start(out=pol_sbs[0], in_=pol_d[0])
            # dummy exp pulls the one-time ACT table load off the critical path
            warm = singles.tile([1, 1], F32)
            nc.vector.memset(warm, 0.0)
            nc.scalar.activation(out=warm, in_=warm, func=EXP, scale=1.0)

            # PE pre-warm: matmuls on uninitialized SBUF garbage (psum never
            # read) keep the HAM clock-gate at 8/8 while the prologue DMAs
            # land, so the first real chains stream at 2.4GHz
            dummy = singles.tile([P, 512], F16, tag="dummy")
            nc.vector.memset(dummy, 0.0)
            for i in range(30):
                ps_w = psQ.tile([P, 512], F32, tag="q", name=f"warm{i}")
                nc.tensor.matmul(
                    ps_w, dummy[:, 0:P], dummy, start=True, stop=True,
                )

            for b in range(1, BL):
                nc.gpsimd.dma_start(out=pol_sbs[b], in_=pol_d[b])

            # ---- persistent v_ext buffers: [ones | v(64)] per head;
            # ones written once, v(64) regions overwritten per batch
            v_exts = []
            for i in range(2):
                ve = singles.tile([P, NT, H * VS], F16, tag=f"ve{i}")
                vv = ve.rearrange("p t (h s) -> p t h s", s=VS)
                # cols [1, VOFF) are never read back from psum -> no zeroing
                nc.vector.memset(vv[:, :, :, 0:1], 1.0)
                v_exts.append(ve)

            # ---- blend tiles: blend[p, t, m] = 1 if m == t*128+p else pol[p]
            # Built lazily (one tile at a time, interleaved into the prior
            # batch's iterations) so the scalar FIFO stays clear early on.
            blends = {}

            def build_blend(b, t):
                if b not in blends:
                    blends[b] = xin.tile([P, NT, N], F16, tag="blend",
                                         name=f"bl{b}")
                blend = blends[b]
                # input is only a shape-provider (scale=0): use xT_sbs[0]
                # which is always loaded first
                nc.scalar.activation(
                    out=blend[:, t, :], in_=xT_sbs[0][:, 0, :],
                    func=IDENT, bias=pol_sbs[b][:, t : t + 1], scale=0.0,
                )
                nc.gpsimd.affine_select(
                    out=blend[:, t, :], in_=blend[:, t, :],
                    compare_op=NE, fill=1.0, base=t * P,
                    pattern=[[-1, N]], channel_multiplier=1,
                )

            # ================= emitters =================
            qkTs = {}   # (b, jt) -> tile

            def qk_chain(b, jt):
                t = mid.tile([P, N], F16, tag=f"qkT{jt}", name=f"qk{b}_{jt}")
                qkTs[(b, jt)] = t
                ps = psQ.tile([P, 512], F32, tag="q")
                half = wq_t if jt < JQK // 2 else wk_t
                joff = (jt % (JQK // 2)) * P
                wtile = half[joff // 384]
                for kt in range(KT):
                    nc.tensor.matmul(
                        ps[:, :N],
                        wtile[:, kt, joff % 384 : joff % 384 + P],
                        xT_sbs[b][:, kt, :],
                        start=(kt == 0), stop=(kt == KT - 1),
                    )
                # bias add + fp16 round (psum -> sbuf), alternating engines
                if jt % 2 == 0:
                    nc.scalar.activation(
                        out=t, in_=ps[:, :N],
                        func=IDENT, bias=bqk_sb[:, jt : jt + 1], scale=1.0,
                    )
                else:
                    nc.vector.tensor_scalar(
                        out=t, in0=ps[:, :N],
                        scalar1=bqk_sb[:, jt : jt + 1], scalar2=None,
                        op0=ADD,
                    )

            def v_chain(b, i):
                nt, c0 = i // 2, (i % 2) * 384
                cw = 384
                v_ext = v_exts[b % 2]
                v_dst = v_ext.rearrange("p t (h s) -> p t h s", s=VS)
                ps = psQ.tile([P, 512], F32, tag="q")
                for kt in range(KT):
                    nc.tensor.matmul(
                        ps[:, :cw],
                        xT_sbs[b][:, kt, nt * P : (nt + 1) * P],
                        wv_t[c0 // 384][:, kt, :],
                        start=(kt == 0), stop=(kt == KT - 1),
                    )
                h0, hn = c0 // HD, cw // HD
                nc.vector.tensor_tensor(
                    out=v_dst[:, nt, h0 : h0 + hn, VOFF : VOFF + HD],
                    in0=ps[:, :cw].rearrange("p (h d) -> p h d", d=HD),
                    in1=bv_sb[:, c0 : c0 + cw].rearrange(
                        "p (h d) -> p h d", d=HD
                    ),
                    op=ADD,
                )

            def s_head(b, h):
                # S^T matmuls for head h -> 3 psum tiles [128 keys, 384 q]
                jq, jk = h // 2, JQK // 2 + h // 2
                base = (h % 2) * HD
                qh = qkTs[(b, jq)][base : base + HD, :]
                kh = qkTs[(b, jk)][base : base + HD, :]
                out = []
                for mt in range(NT):
                    ps_s = psS.tile([P, 512], F32, tag="s")
                    nc.tensor.matmul(
                        ps_s[:, :N],
                        kh[:, mt * P : (mt + 1) * P],
                        qh,
                        start=True, stop=True,
                    )
                    out.append(ps_s)
                return out

            def exp_blend(b, h, ps_ss):
                # ea = exp(S*scale) into one fused [P, NT, N] tile, then a
                # single wide DVE multiply applies the blend mask
                ea = eact.tile([P, NT, N], F16, tag="ea", name=f"ea{h}")
                for mt in range(NT):
                    nc.scalar.activation(
                        out=ea[:, mt, :], in_=ps_ss[mt][:, :N],
                        func=EXP, scale=SCALE,
                    )
                eh = ehatp.tile([P, NT, N], F16, tag="eh", name=f"eh{h}")
                nc.vector.tensor_tensor(
                    out=eh, in0=ea, in1=blends[b], op=MULT,
                )
                return eh

            def av_head(b, h, ehat):
                v_ext = v_exts[b % 2]
                ps_av = psAV.tile([P, 512], F32, tag="av", name=f"av{h}")
                for mt in range(NT):
                    nc.tensor.matmul(
                        ps_av[: VOFF + HD, :N],
                        v_ext[:, mt, h * VS : (h + 1) * VS],
                        ehat[:, mt, :],
                        start=(mt == 0), stop=(mt == NT - 1),
                    )
                return ps_av

            def norm_head(h, ps_av, oT):
                # row-sum sits in psum partition 0 via the ones column;
                # 1/sum read directly from psum (saves an ACT copy)
                base = (h % 2) * HD
                jq = h // 2
                r_sb = small.tile([1, N], F32, tag="r", name=f"r{h}")
                nc.vector.reciprocal_approx_fast(out=r_sb, in_=ps_av[0:1, :N])
                rb_sb = small.tile([HD, N], F32, tag="rb", name=f"rb{h}")
                nc.gpsimd.partition_broadcast(rb_sb, r_sb)
                nc.vector.tensor_tensor(
                    out=oT[jq][base : base + HD, :],
                    in0=ps_av[VOFF : VOFF + HD, :N], in1=rb_sb, op=MULT,
                )

            def proj_chain(b, i, oT):
                nt, c0 = i // 2, (i % 2) * 384
                cw = 384
                out_sb = proj_sbs[b % 2]
                out_v = out_d[b].rearrange("(t p) o -> p t o", p=P)
                ps = psQ.tile([P, 512], F32, tag="q")
                for kt in range(KT):
                    nc.tensor.matmul(
                        ps[:, :cw],
                        oT[kt][:, nt * P : (nt + 1) * P],
                        wp_t[c0 // 384][:, kt, :],
                        start=(kt == 0), stop=(kt == KT - 1),
                    )
                nc.vector.tensor_add(
                    out=out_sb[:, nt, c0 : c0 + cw],
                    in0=ps[:, :cw],
                    in1=bp_sb[:, c0 : c0 + cw],
                )
                eng = (nc.sync, nc.scalar, nc.gpsimd)[nt]
                eng.dma_start(
                    out=out_v[:, nt : nt + 1, c0 : c0 + cw],
                    in_=out_sb[:, nt : nt + 1, c0 : c0 + cw],
                )

            proj_sbs = [outp.tile([P, NT, C], F32, tag="out", name=f"os{i}")
                        for i in range(2)]

            # ================= schedule =================
            # prologue: batch 0's qk and v chains, with the bulk input
            # transfers issued once the critical loads are in flight
            nc.gpsimd.dma_start(out=xT_sbs[1], in_=xT_d[1])
            for jt in (0, 6, 1, 7, 2, 8, 3, 9):
                qk_chain(0, jt)
                if jt == 6:
                    dma_w(nc.scalar, wv_t, 0, (4, 5))
                    dma_w(nc.scalar, wv_t, 1, (4, 5))
            for t in range(NT):
                build_blend(0, t)
            nc.gpsimd.dma_start(
                out=meta_sb[:, JQK:], in_=meta_d[:, JQK:]
            )
            for i in (0, 2, 4):
                v_chain(0, i)

            oTs = {}

            def make_oT(b):
                oTs[b] = [
                    mid.tile([P, N], F16, tag=f"oT{kt}", name=f"oT{b}_{kt}")
                    for kt in range(KT)
                ]

            make_oT(0)
            pending = s_head(0, 0)
            late = None

            for b in range(BL):
                oT = oTs[b]
                # filler chains, placed by deadline: this batch's deferred
                # tail chains first, then b-1's output projection, then the
                # head chains of b+1.  This keeps the last batch supplied
                # with PE work instead of front-loading everything.
                per_iter = [[] for _ in range(H)]
                per_iter[0] = [("qk", b, 4), ("qk", b, 10)]
                per_iter[1] = [("qk", b, 5), ("qk", b, 11)]
                per_iter[2] = [("v", b, 1)]
                per_iter[3] = [("v", b, 3)]
                per_iter[4] = [("v", b, 5)]
                if b > 0:
                    for i, it in enumerate((2, 3, 4, 5, 6, 7)):
                        per_iter[it].append(("p", b - 1, i))
                if b + 1 < BL:
                    per_iter[5].append(("qk", b + 1, 0))
                    per_iter[6].append(("qk", b + 1, 6))
                    per_iter[7].append(("qk", b + 1, 1))
                    per_iter[8] += [("qk", b + 1, 7), ("qk", b + 1, 2)]
                    per_iter[9] += [("qk", b + 1, 8), ("qk", b + 1, 3)]
                    per_iter[10] += [("qk", b + 1, 9), ("v", b + 1, 0)]
                    per_iter[11] += [("v", b + 1, 2), ("v", b + 1, 4)]

                def emit_filler(f):
                    kind, fb, fi = f
                    if kind == "qk":
                        qk_chain(fb, fi)
                    elif kind == "v":
                        v_chain(fb, fi)
                    else:
                        proj_chain(fb, fi, oTs[fb])

                for h in range(H):
                    # stagger the remaining bulk loads through batch 0
                    if b == 0:
                        if h == 1:
                            nc.sync.dma_start(out=xT_sbs[2], in_=xT_d[2])
                        elif h == 3:
                            dma_w(nc.sync, wp_t, 0, (6, 7))
                            dma_w(nc.sync, wp_t, 1, (6, 7))
                        elif h == 5:
                            nc.sync.dma_start(out=xT_sbs[3], in_=xT_d[3])
                    # build next batch's blend tiles mid-batch
                    if b + 1 < BL and h in (3, 5, 7):
                        build_blend(b + 1, (h - 3) // 2)
                    # 1) softmax element-wise ops for head h (ACT + one DVE)
                    ehat = exp_blend(b, h, pending)
                    # 2) next head's S matmuls (PE; psS pool)
                    if h + 1 < H:
                        pending = s_head(b, h + 1)
                    elif b + 1 < BL:
                        make_oT(b + 1)
                        pending = s_head(b + 1, 0)
                    # 3) filler chains (PE; psQ pool) hide the ehat latency
                    fl = per_iter[h]
                    if fl:
                        emit_filler(fl[0])
                    # 4) attn@v matmuls for head h (PE; psAV pool)
                    ps_av = av_head(b, h, ehat)
                    # 5) late softmax stages of the PREVIOUS head (1-head
                    #    skew keeps recip/rnorm waits off the FIFO heads);
                    #    no skew at the very end to shorten the tail
                    if late:
                        norm_head(*late)
                    if b == BL - 1 and h == H - 1:
                        norm_head(h, ps_av, oT)
                        late = None
                    else:
                        late = (h, ps_av, oT)
                    for f in fl[1:]:
                        emit_filler(f)

            if late:
                norm_head(*late)
            # final batch's output projection
            for i in range(6):
                proj_chain(BL - 1, i, oTs[BL - 1])

    nc.compile()
    return nc


def _get_nc():
    if "nc" not in _CACHE:
        _CACHE["nc"] = _build_nc()
    return _CACHE["nc"]


def kernel(x, policy, qkv_w, qkv_b, proj_w, proj_b):
    from concourse.bass_utils import run_bass_kernel_spmd

    nc = _get_nc()

    x = np.asarray(x, dtype=np.float32)
    policy = np.asarray(policy, dtype=np.float32)
    qkv_w = np.asarray(qkv_w, dtype=np.float32)
    qkv_b = np.asarray(qkv_b, dtype=np.float32)
    proj_w = np.asarray(proj_w, dtype=np.float32)
    proj_b = np.asarray(proj_b, dtype=np.float32)

    xT = np.ascontiguousarray(
        x.transpose(0, 2, 1).reshape(B, KT, P, N).transpose(0, 2, 1, 3)
    ).astype(np.float16)  # [B, P, KT, N]
    pol = np.ascontiguousarray(
        policy.reshape(B, N).reshape(B, NT, P).transpose(0, 2, 1)
    )  # [B, P, NT]

    def to_sbuf_layout(w):  # [C, J] -> [P, KT, J]
        return np.ascontiguousarray(w.reshape(KT, P, -1).transpose(1, 0, 2))

    wqkT = to_sbuf_layout(qkv_w[: 2 * C].T.astype(np.float16))
    wpT = to_sbuf_layout(proj_w.T.astype(np.float16))
    bqk = np.ascontiguousarray(qkv_b[: 2 * C].reshape(JQK, P).T)  # [P, 12]
    wvT = to_sbuf_layout(qkv_w[2 * C :].T.astype(np.float16))
    # 8 kt-major sections [P, KT, 384]: wq0 wk0 wq1 wk1 wv0 wv1 wp0 wp1
    wall = np.ascontiguousarray(np.stack([
        wqkT[:, :, 0:384], wqkT[:, :, 768:1152],
        wqkT[:, :, 384:768], wqkT[:, :, 1152:1536],
        wvT[:, :, 0:384], wvT[:, :, 384:768],
        wpT[:, :, 0:384], wpT[:, :, 384:768],
    ], axis=1))  # [P, 8, KT, 384]
    # bqk | bv | bp broadcast rows in one f32 tensor [P, 12 + 2C]
    meta = np.ascontiguousarray(np.concatenate([
        bqk,
        np.broadcast_to(qkv_b[2 * C :], (P, C)),
        np.broadcast_to(proj_b, (P, C)),
    ], axis=1).astype(np.float32))

    in_maps = []
    for c in range(NCORES):
        s = slice(c * BL, (c + 1) * BL)
        in_maps.append({
            "xT": xT[s], "pol": pol[s],
            "wall": wall, "meta": meta,
        })

    res = run_bass_kernel_spmd(nc, in_maps, core_ids=list(range(NCORES)))
    _CACHE["last_results"] = res
    out = np.concatenate(
        [res.results[c]["out"] for c in range(NCORES)], axis=0
    ).astype(np.float32)
    return out
